# revision 16
# baseline (speedup 1.0000x reference)
"""Trainium2 Bass kernel for nn_Decoder (VQ codebook decoder), fused-conv version.

Pipeline (per batch b): gather codebook entries by index, scale, sum over
quantizers, per-group linear projection -> zc [1024, L]; ConvTranspose1d
(1024->1024, k=4, s=2, SAME) -> zu; Conv1d (1024->512, k=7, SAME) -> x.

Key optimization: the upsample and head conv are linear maps, so they are
fused on the host into a 5-tap conv directly on zc, split by output parity:
  x[2n]   = sum_{j=-2..2} E_j zc[n+j] + b_fused
  x[2n+1] = sum_{j=-2..2} O_j zc[n+j] + b_fused
with E_j/O_j = sums of W_head[:,:,k] @ W_up[:,:,q]^T products ([512,1024]
each) and b_fused = b_head + sum_k W_head[:,:,k] b_up. This halves tensor
engine FLOPs (21.5 GF vs 47.3 GF per core) and removes the ze/zo HBM
round-trip.

The fused interior formula is exact except where the head conv window
touches zu frames outside [0, 2L): extending the upsample formula leaks
zu[-1] = W3^T zc[0] and zu[2L] = W0^T zc[L-1] (all other out-of-range zu
frames evaluate to 0 because zc[m]=0 outside [0,L), enforced on-device by
gathering a zero column for out-of-range indices). The 3 leftmost / 3
rightmost output columns per sequence are corrected exactly on the host,
including the (position-dependent) b_up validity terms.

Sharding: 8 cores = (batch b in 0..3) x (half of L). Each core computes
x[b, :, half*4096 : (half+1)*4096] from zc frames [l0-3, l1+2) (halo
recomputed locally; gathers are chunked and pipelined against the
projection matmuls and the fused-conv matmuls).
"""
import numpy as np
import ml_dtypes

import concourse.mybir as mybir
import concourse.tile as tile
from concourse import bacc
from concourse.bass_utils import run_bass_kernel_spmd

# structural constants (hardcoded per contest contract)
G, Q, C, D = 2, 8, 1024, 8
DIMS, DPG = 1024, 512
B, L = 4, 4096
HEAD_OUT = 512
LLOC = 2048          # frames per core
EFR = 2053           # zc frames [l0-3, l1+2)
NIDX = 2064          # gather count (mult of 16, >= EFR)

f32 = mybir.dt.float32
bf16 = mybir.dt.bfloat16
i16 = mybir.dt.int16

DT_CONV = "bf16"

# stage A psum chunks (<=512 cols per PSUM bank); conv chunks of 512.
# Finer chunks at the head so the first conv chunk can start early.
A_CH = [(0, 256), (256, 256), (512, 256), (768, 256), (1024, 256),
        (1280, 256), (1536, 256), (1792, 256), (2048, 5)]
# gather chunks in idx-columns (16 indices per col); X cols = 16*j0..16*(j0+nj)
G_CH = [(0, 16), (16, 16), (32, 16), (48, 16), (64, 32), (96, 33)]


# ---------------------------------------------------------------- host prep
def prep_shared(codebooks, scales, W_out, b_out, W_up, b_up, W_head, b_head):
    """Core-independent packed arrays (f32 gather table, bf16 weights)."""
    cbT = np.zeros((G, 128, C + 1), np.float32)
    W2 = np.zeros((G, 128, DPG), np.float32)
    for g in range(G):
        for q in range(Q):
            cbT[g, 16 * q : 16 * q + 8, :C] = codebooks[g, q].T
            cbT[g, 16 * q + 8, :C] = 1.0
            W2[g, 16 * q : 16 * q + 8] = scales[g, q][:, None] * W_out[g]
        W2[g, 8] = b_out[g]  # bias carrier: q=0's ones-row only
    W2pack = np.empty((128, G * DPG), np.float32)
    for g in range(G):
        W2pack[:, g * DPG : (g + 1) * DPG] = W2[g]

    # fused conv matrices: x[2n+p] = sum_j M[p,j] zc[n+j] + bfused
    W = [W_up[:, :, k].T.astype(np.float32) for k in range(4)]  # [zu, zc]
    H = [W_head[:, :, k].astype(np.float32) for k in range(7)]  # [x, zu]
    E = [
        H[0] @ W[1] + H[1] @ W[0],
        H[0] @ W[3] + H[1] @ W[2] + H[2] @ W[1] + H[3] @ W[0],
        H[2] @ W[3] + H[3] @ W[2] + H[4] @ W[1] + H[5] @ W[0],
        H[4] @ W[3] + H[5] @ W[2] + H[6] @ W[1],
        H[6] @ W[3],
    ]
    O = [
        H[0] @ W[0],
        H[0] @ W[2] + H[1] @ W[1] + H[2] @ W[0],
        H[1] @ W[3] + H[2] @ W[2] + H[3] @ W[1] + H[4] @ W[0],
        H[3] @ W[3] + H[4] @ W[2] + H[5] @ W[1] + H[6] @ W[0],
        H[5] @ W[3] + H[6] @ W[2],
    ]
    bfused = b_head + sum(H[k] @ b_up for k in range(7))
    # Mpack[ci_lo, co_t, ci_t, p, j, co_lo] = M[p][j][co_t*128+co_lo, ci_t*128+ci_lo]
    # (co_t outermost so each co-tile's weights are one contiguous DMA)
    Mpack = np.empty((128, 4, 8, 2, 5, 128), np.float32)
    for p, mats in enumerate((E, O)):
        for j in range(5):
            m4 = mats[j].reshape(4, 128, 8, 128)  # [co_t, co_lo, ci_t, ci_lo]
            Mpack[:, :, :, p, j, :] = m4.transpose(3, 0, 2, 1)
    bhead = bfused.reshape(4, 128).T  # [128, co_t]
    return dict(
        cbT=cbT.astype(np.float32),
        W2=W2pack.astype(ml_dtypes.bfloat16),
        M=Mpack.astype(ml_dtypes.bfloat16),
        bh=bhead.astype(np.float32),
    )


def prep_core(indices, core):
    """Per-core packed index array."""
    b, half = core // 2, core % 2
    l0 = half * LLOC
    ls = np.arange(NIDX) + l0 - 3
    valid = (ls >= 0) & (ls < L)
    lc = np.clip(ls, 0, L - 1)
    idx16 = np.empty((G, 128, NIDX // 16), np.int16)
    for g in range(G):
        for q in range(Q):
            v = indices[b, g * Q + q, lc]
            stream = np.where(valid & (v >= 0), v, C).astype(np.int16)
            idx16[g, 16 * q : 16 * q + 16] = stream.reshape(NIDX // 16, 16).T
    return idx16


def host_corrections(indices, codebooks, scales, W_out, b_out, W_up, b_up,
                     W_head, b_head):
    """Exact fix-ups for the 3 leftmost/rightmost output columns per batch.

    Returns (left [B, 512, 3], right [B, 512, 3]) to SUBTRACT from the
    assembled output at columns 0..2 and 2L-3..2L-1.
    """
    ii = indices.reshape(B, G, Q, L)
    H = [W_head[:, :, k].astype(np.float32) for k in range(7)]

    def zc_frame(b, l):
        zs = []
        for g in range(G):
            acc = np.zeros(D, np.float32)
            for q in range(Q):
                v = int(ii[b, g, q, l])
                if v >= 0:
                    acc += codebooks[g, q, v] * scales[g, q]
            zs.append(acc @ W_out[g] + b_out[g])
        return np.concatenate(zs)  # [1024]

    left = np.zeros((B, HEAD_OUT, 3), np.float32)
    right = np.zeros((B, HEAD_OUT, 3), np.float32)
    for b in range(B):
        vleft = W_up[:, :, 3].T @ zc_frame(b, 0)      # zu_ext[-1]
        vright = W_up[:, :, 0].T @ zc_frame(b, L - 1)  # zu_ext[2L]
        for t in range(3):
            hb = sum(H[k] for k in range(0, 3 - t)) @ b_up  # invalid f<0
            left[b, :, t] = H[2 - t] @ vleft + hb
        for s, t in enumerate(range(2 * L - 3, 2 * L)):
            hb = sum(H[k] for k in range(2 * L + 3 - t, 7)) @ b_up  # f>=2L
            right[b, :, s] = H[2 * L + 3 - t] @ vright + hb
    return left, right


# ---------------------------------------------------------------- bass build
def build_nc(dt_conv=DT_CONV, reps=1):
    dt = bf16
    nc = bacc.Bacc("TRN2", target_bir_lowering=False, debug=False)

    cbT_d = nc.dram_tensor("cbT", [G, 128, C + 1], f32, kind="ExternalInput")
    idx_d = nc.dram_tensor("idx", [G, 128, NIDX // 16], i16, kind="ExternalInput")
    W2_d = nc.dram_tensor("W2", [128, G * DPG], dt, kind="ExternalInput")
    M_d = nc.dram_tensor("M", [128, 4, 8, 2, 5, 128], dt, kind="ExternalInput")
    bh_d = nc.dram_tensor("bh", [128, 4], f32, kind="ExternalInput")
    x_d = nc.dram_tensor("x", [HEAD_OUT, 2 * LLOC], f32, kind="ExternalOutput")

    with tile.TileContext(nc) as tc:
      for rep in range(reps):
        with (
            tc.tile_pool(name=f"p1c{rep}", bufs=1) as p1c,
            tc.tile_pool(name=f"p2x{rep}", bufs=3) as p2x,
            tc.tile_pool(name=f"psA{rep}", bufs=2, space="PSUM") as psA,
            tc.tile_pool(name=f"psC{rep}", bufs=4, space="PSUM") as psC,
        ):
            cbT = [p1c.tile([128, C + 1], f32, name=f"cbT{g}") for g in range(G)]
            idxs = [p1c.tile([128, NIDX // 16], i16, name=f"idxs{g}") for g in range(G)]
            W2t = p1c.tile([128, G * DPG], dt, name="W2t")
            # per-gather-chunk tiles so downstream deps are exact
            X = [[p1c.tile([128, 16 * nj], f32, name=f"X{g}_{ci}")
                  for ci, (j0, nj) in enumerate(G_CH)] for g in range(G)]
            Xb = [[p1c.tile([128, 16 * nj], dt, name=f"Xb{g}_{ci}")
                   for ci, (j0, nj) in enumerate(G_CH)] for g in range(G)]
            zc = p1c.tile([128, 8, EFR], dt, name="zc")
            Mt = p1c.tile([128, 4, 8, 2, 5, 128], dt, name="Mt")
            bh = p1c.tile([128, 4], f32, name="bh")

            # idx/table first: the gathers are the serial critical producer;
            # the big fused-weight DMA queues behind them (needed ~35us in).
            for g in range(G):
                nc.sync.dma_start(out=idxs[g][:], in_=idx_d.ap()[g])
                nc.sync.dma_start(out=cbT[g][:], in_=cbT_d.ap()[g])
            nc.sync.dma_start(out=W2t[:], in_=W2_d.ap())
            nc.sync.dma_start(out=bh[:], in_=bh_d.ap())
            for co_t in range(4):
                nc.sync.dma_start(out=Mt[:, co_t], in_=M_d.ap()[:, co_t])

            def gather_chunk(ci):
                j0, nj = G_CH[ci]
                nx = 16 * nj
                for g in range(G):
                    nc.gpsimd.ap_gather(
                        X[g][ci][:],
                        cbT[g][:],
                        idxs[g][:, j0 : j0 + nj],
                        channels=128,
                        num_elems=C + 1,
                        d=1,
                        num_idxs=nx,
                    )
                    nc.vector.tensor_copy(Xb[g][ci][:], X[g][ci][:])

            def a_chunk(ai):
                c0, N = A_CH[ai]
                # locate covering gather chunk (A chunks never straddle)
                for ci, (j0, nj) in enumerate(G_CH):
                    if 16 * j0 <= c0 and c0 + N <= 16 * (j0 + nj):
                        off = c0 - 16 * j0
                        break
                for g in range(G):
                    for et in range(4):
                        ct = g * 4 + et
                        ps = psA.tile([128, N], f32, tag="psA")
                        nc.tensor.matmul(
                            ps[:],
                            lhsT=W2t[:, g * DPG + et * 128 : g * DPG + et * 128 + 128],
                            rhs=Xb[g][ci][:, off : off + N],
                            start=True,
                            stop=True,
                        )
                        nc.scalar.copy(zc[:, ct, c0 : c0 + N], ps[:])

            def conv_chunk(k):
                base = 512 * k + 1
                for co_t in range(4):
                    xt = p2x.tile([128, 1024], f32, tag="xt")
                    xv = xt[:].rearrange("p (n two) -> p two n", two=2)
                    for par in range(2):
                        ps = psC.tile([128, 512], f32, tag="psC")
                        for jj in range(5):
                            for ci_t in range(8):
                                nc.tensor.matmul(
                                    ps[:],
                                    lhsT=Mt[:, co_t, ci_t, par, jj],
                                    rhs=zc[:, ci_t, base + jj : base + jj + 512],
                                    start=(jj == 0 and ci_t == 0),
                                    stop=(jj == 4 and ci_t == 7),
                                )
                        nc.vector.tensor_scalar_add(
                            xv[:, par], ps[:], bh[:, co_t : co_t + 1]
                        )
                    nc.scalar.dma_start(
                        out=x_d.ap()[
                            co_t * 128 : co_t * 128 + 128,
                            1024 * k : 1024 * k + 1024,
                        ],
                        in_=xt[:],
                    )

            # All gathers emitted first (lowest scheduler priority) so the
            # serial gpsimd stream runs chunk-major/group-interleaved and
            # never reorders behind tensor work; tensor work is emitted in
            # consumption order. Tile inserts slice-level deps.
            for ci in range(len(G_CH)):
                gather_chunk(ci)
            a_chunk(0)
            a_chunk(1)
            a_chunk(2)
            conv_chunk(0)
            a_chunk(3)
            a_chunk(4)
            conv_chunk(1)
            a_chunk(5)
            a_chunk(6)
            conv_chunk(2)
            a_chunk(7)
            a_chunk(8)
            conv_chunk(3)
    nc.compile()
    return nc


# ---------------------------------------------------------------- entry
_CACHE = {}
_LAST_CORR = None


def _get_nc(dt_conv):
    if dt_conv not in _CACHE:
        _CACHE[dt_conv] = build_nc(dt_conv)
    return _CACHE[dt_conv]


def make_in_maps(inputs, dt_conv=DT_CONV):
    global _LAST_CORR
    codebooks = np.asarray(inputs["codebooks"], np.float32)
    scales = np.asarray(inputs["scales"], np.float32)
    W_out = np.asarray(inputs["W_out"], np.float32)
    b_out = np.asarray(inputs["b_out"], np.float32)
    W_up = np.asarray(inputs["W_up"], np.float32)
    b_up = np.asarray(inputs["b_up"], np.float32)
    W_head = np.asarray(inputs["W_head"], np.float32)
    b_head = np.asarray(inputs["b_head"], np.float32)
    indices = np.asarray(inputs["indices"])
    shared = prep_shared(codebooks, scales, W_out, b_out, W_up, b_up,
                         W_head, b_head)
    _LAST_CORR = host_corrections(indices, codebooks, scales, W_out, b_out,
                                  W_up, b_up, W_head, b_head)
    in_maps = []
    for core in range(8):
        m = dict(shared)
        m["idx"] = prep_core(indices, core)
        in_maps.append(m)
    return in_maps


def assemble(results, corr=None):
    corr = corr if corr is not None else _LAST_CORR
    out = np.empty((B, HEAD_OUT, 2 * L), np.float32)
    for core in range(8):
        b, half = core // 2, core % 2
        out[b, :, half * 2 * LLOC : (half + 1) * 2 * LLOC] = results[core]["x"]
    if corr is not None:
        left, right = corr
        out[:, :, 0:3] -= left
        out[:, :, 2 * L - 3 : 2 * L] -= right
    return out


def kernel(**inputs):
    nc = _get_nc(DT_CONV)
    in_maps = make_in_maps(inputs, DT_CONV)
    res = run_bass_kernel_spmd(nc, in_maps, list(range(8)))
    return assemble(res.results, _LAST_CORR)


# revision 19
# speedup vs baseline: 1.0992x; 1.0992x over previous
"""Trainium2 Bass kernel for nn_Decoder (VQ codebook decoder), fused-conv version.

Pipeline (per batch b): gather codebook entries by index, scale, sum over
quantizers, per-group linear projection -> zc [1024, L]; ConvTranspose1d
(1024->1024, k=4, s=2, SAME) -> zu; Conv1d (1024->512, k=7, SAME) -> x.

Key optimization: the upsample and head conv are linear maps, so they are
fused on the host into a 5-tap conv directly on zc, split by output parity:
  x[2n]   = sum_{j=-2..2} E_j zc[n+j] + b_fused
  x[2n+1] = sum_{j=-2..2} O_j zc[n+j] + b_fused
with E_j/O_j = sums of W_head[:,:,k] @ W_up[:,:,q]^T products ([512,1024]
each) and b_fused = b_head + sum_k W_head[:,:,k] b_up. This halves tensor
engine FLOPs (21.5 GF vs 47.3 GF per core) and removes the ze/zo HBM
round-trip.

The fused interior formula is exact except where the head conv window
touches zu frames outside [0, 2L): extending the upsample formula leaks
zu[-1] = W3^T zc[0] and zu[2L] = W0^T zc[L-1] (all other out-of-range zu
frames evaluate to 0 because zc[m]=0 outside [0,L), enforced on-device by
gathering a zero column for out-of-range indices). The 3 leftmost / 3
rightmost output columns per sequence are corrected exactly on the host,
including the (position-dependent) b_up validity terms.

Sharding: 8 cores = (batch b in 0..3) x (half of L). Each core computes
x[b, :, half*4096 : (half+1)*4096] from zc frames [l0-3, l1+2) (halo
recomputed locally; gathers are chunked and pipelined against the
projection matmuls and the fused-conv matmuls).
"""
import numpy as np
import ml_dtypes

import concourse.mybir as mybir
import concourse.tile as tile
from concourse import bacc
from concourse.bass_utils import run_bass_kernel_spmd

# structural constants (hardcoded per contest contract)
G, Q, C, D = 2, 8, 1024, 8
DIMS, DPG = 1024, 512
B, L = 4, 4096
HEAD_OUT = 512
LLOC = 2048          # frames per core
EFR = 2053           # zc frames [l0-3, l1+2)
NIDX = 2064          # gather count (mult of 16, >= EFR)

f32 = mybir.dt.float32
bf16 = mybir.dt.bfloat16
i16 = mybir.dt.int16

DT_CONV = "bf16"

# stage A psum chunks (<=512 cols per PSUM bank); conv chunks of 512.
# Finer chunks at the head so the first conv chunk can start early.
A_CH = [(0, 256), (256, 256), (512, 256), (768, 256), (1024, 256),
        (1280, 256), (1536, 256), (1792, 256), (2048, 5)]
# gather chunks in idx-columns (16 indices per col); X cols = 16*j0..16*(j0+nj)
G_CH = [(0, 16), (16, 16), (32, 16), (48, 16), (64, 32), (96, 33)]


# ---------------------------------------------------------------- host prep
def prep_shared(codebooks, scales, W_out, b_out, W_up, b_up, W_head, b_head):
    """Core-independent packed arrays (f32 gather table, bf16 weights)."""
    cbT = np.zeros((G, 128, C + 1), np.float32)
    W2 = np.zeros((G, 128, DPG), np.float32)
    for g in range(G):
        for q in range(Q):
            cbT[g, 16 * q : 16 * q + 8, :C] = codebooks[g, q].T
            cbT[g, 16 * q + 8, :C] = 1.0
            W2[g, 16 * q : 16 * q + 8] = scales[g, q][:, None] * W_out[g]
        W2[g, 8] = b_out[g]  # bias carrier: q=0's ones-row only
    W2pack = np.empty((128, G * DPG), np.float32)
    for g in range(G):
        W2pack[:, g * DPG : (g + 1) * DPG] = W2[g]

    # fused conv matrices: x[2n+p] = sum_j M[p,j] zc[n+j] + bfused
    W = [W_up[:, :, k].T.astype(np.float32) for k in range(4)]  # [zu, zc]
    H = [W_head[:, :, k].astype(np.float32) for k in range(7)]  # [x, zu]
    E = [
        H[0] @ W[1] + H[1] @ W[0],
        H[0] @ W[3] + H[1] @ W[2] + H[2] @ W[1] + H[3] @ W[0],
        H[2] @ W[3] + H[3] @ W[2] + H[4] @ W[1] + H[5] @ W[0],
        H[4] @ W[3] + H[5] @ W[2] + H[6] @ W[1],
        H[6] @ W[3],
    ]
    O = [
        H[0] @ W[0],
        H[0] @ W[2] + H[1] @ W[1] + H[2] @ W[0],
        H[1] @ W[3] + H[2] @ W[2] + H[3] @ W[1] + H[4] @ W[0],
        H[3] @ W[3] + H[4] @ W[2] + H[5] @ W[1] + H[6] @ W[0],
        H[5] @ W[3] + H[6] @ W[2],
    ]
    bfused = b_head + sum(H[k] @ b_up for k in range(7))
    # Mpack[ci_lo, co_t, ci_t, p, j, co_lo] = M[p][j][co_t*128+co_lo, ci_t*128+ci_lo]
    # (co_t outermost so each co-tile's weights are one contiguous DMA)
    Mpack = np.empty((128, 4, 8, 2, 5, 128), np.float32)
    for p, mats in enumerate((E, O)):
        for j in range(5):
            m4 = mats[j].reshape(4, 128, 8, 128)  # [co_t, co_lo, ci_t, ci_lo]
            Mpack[:, :, :, p, j, :] = m4.transpose(3, 0, 2, 1)
    bhead = bfused.reshape(4, 128).T  # [128, co_t]
    return dict(
        cbT=cbT.astype(np.float32),
        W2=W2pack.astype(ml_dtypes.bfloat16),
        M=Mpack.astype(ml_dtypes.bfloat16),
        bh=bhead.astype(np.float32),
    )


def prep_core(indices, core):
    """Per-core packed index array."""
    b, half = core // 2, core % 2
    l0 = half * LLOC
    ls = np.arange(NIDX) + l0 - 3
    valid = (ls >= 0) & (ls < L)
    lc = np.clip(ls, 0, L - 1)
    idx16 = np.empty((G, 128, NIDX // 16), np.int16)
    for g in range(G):
        for q in range(Q):
            v = indices[b, g * Q + q, lc]
            stream = np.where(valid & (v >= 0), v, C).astype(np.int16)
            idx16[g, 16 * q : 16 * q + 16] = stream.reshape(NIDX // 16, 16).T
    return idx16


def host_corrections(indices, codebooks, scales, W_out, b_out, W_up, b_up,
                     W_head, b_head):
    """Exact fix-ups for the 3 leftmost/rightmost output columns per batch.

    Returns (left [B, 512, 3], right [B, 512, 3]) to SUBTRACT from the
    assembled output at columns 0..2 and 2L-3..2L-1.
    """
    ii = indices.reshape(B, G, Q, L)
    H = [W_head[:, :, k].astype(np.float32) for k in range(7)]

    def zc_frame(b, l):
        zs = []
        for g in range(G):
            acc = np.zeros(D, np.float32)
            for q in range(Q):
                v = int(ii[b, g, q, l])
                if v >= 0:
                    acc += codebooks[g, q, v] * scales[g, q]
            zs.append(acc @ W_out[g] + b_out[g])
        return np.concatenate(zs)  # [1024]

    left = np.zeros((B, HEAD_OUT, 3), np.float32)
    right = np.zeros((B, HEAD_OUT, 3), np.float32)
    for b in range(B):
        vleft = W_up[:, :, 3].T @ zc_frame(b, 0)      # zu_ext[-1]
        vright = W_up[:, :, 0].T @ zc_frame(b, L - 1)  # zu_ext[2L]
        for t in range(3):
            hb = sum(H[k] for k in range(0, 3 - t)) @ b_up  # invalid f<0
            left[b, :, t] = H[2 - t] @ vleft + hb
        for s, t in enumerate(range(2 * L - 3, 2 * L)):
            hb = sum(H[k] for k in range(2 * L + 3 - t, 7)) @ b_up  # f>=2L
            right[b, :, s] = H[2 * L + 3 - t] @ vright + hb
    return left, right


# ---------------------------------------------------------------- bass build
def build_nc(dt_conv=DT_CONV, reps=1):
    dt = bf16
    nc = bacc.Bacc("TRN2", target_bir_lowering=False, debug=False)

    cbT_d = nc.dram_tensor("cbT", [G, 128, C + 1], f32, kind="ExternalInput")
    idx_d = nc.dram_tensor("idx", [G, 128, NIDX // 16], i16, kind="ExternalInput")
    W2_d = nc.dram_tensor("W2", [128, G * DPG], dt, kind="ExternalInput")
    M_d = nc.dram_tensor("M", [128, 4, 8, 2, 5, 128], dt, kind="ExternalInput")
    bh_d = nc.dram_tensor("bh", [128, 4], f32, kind="ExternalInput")
    x_d = nc.dram_tensor("x", [HEAD_OUT, 2 * LLOC], f32, kind="ExternalOutput")

    with tile.TileContext(nc) as tc:
      for rep in range(reps):
        with (
            tc.tile_pool(name=f"p1c{rep}", bufs=1) as p1c,
            tc.tile_pool(name=f"p2x{rep}", bufs=3) as p2x,
            tc.tile_pool(name=f"psA{rep}", bufs=2, space="PSUM") as psA,
            tc.tile_pool(name=f"psC{rep}", bufs=4, space="PSUM") as psC,
        ):
            cbT = [p1c.tile([128, C + 1], f32, name=f"cbT{g}") for g in range(G)]
            idxs = [p1c.tile([128, NIDX // 16], i16, name=f"idxs{g}") for g in range(G)]
            W2t = p1c.tile([128, G * DPG], dt, name="W2t")
            X = [p1c.tile([128, NIDX], f32, name=f"X{g}") for g in range(G)]
            Xb = [p1c.tile([128, NIDX], dt, name=f"Xb{g}") for g in range(G)]
            zc = p1c.tile([128, 8, EFR], dt, name="zc")
            Mt = p1c.tile([128, 4, 8, 2, 5, 128], dt, name="Mt")
            bh = p1c.tile([128, 4], f32, name="bh")

            # idx/table first: the gathers are the serial critical producer;
            # the big fused-weight DMA queues behind them (needed ~35us in).
            for g in range(G):
                nc.sync.dma_start(out=idxs[g][:], in_=idx_d.ap()[g])
                nc.sync.dma_start(out=cbT[g][:], in_=cbT_d.ap()[g])
            nc.sync.dma_start(out=W2t[:], in_=W2_d.ap())
            nc.sync.dma_start(out=bh[:], in_=bh_d.ap())
            for co_t in range(4):
                nc.sync.dma_start(out=Mt[:, co_t], in_=M_d.ap()[:, co_t])

            def gather_chunk(ci):
                j0, nj = G_CH[ci]
                x0, nx = 16 * j0, 16 * nj
                for g in range(G):
                    nc.gpsimd.ap_gather(
                        X[g][:, x0 : x0 + nx],
                        cbT[g][:],
                        idxs[g][:, j0 : j0 + nj],
                        channels=128,
                        num_elems=C + 1,
                        d=1,
                        num_idxs=nx,
                    )

            def cast_chunk(ci):
                # emitted in consumption order, NOT with the gathers: engines
                # run their program in-order, so a cast waiting on a late
                # gather must not sit ahead of conv psum drains in the
                # vector queue (it would stall the psC ring -> tensor).
                j0, nj = G_CH[ci]
                x0, nx = 16 * j0, 16 * nj
                for g in range(G):
                    nc.vector.tensor_copy(
                        Xb[g][:, x0 : x0 + nx], X[g][:, x0 : x0 + nx]
                    )

            def a_chunk(ai):
                c0, N = A_CH[ai]
                for g in range(G):
                    for et in range(4):
                        ct = g * 4 + et
                        ps = psA.tile([128, N], f32, tag="psA")
                        nc.tensor.matmul(
                            ps[:],
                            lhsT=W2t[:, g * DPG + et * 128 : g * DPG + et * 128 + 128],
                            rhs=Xb[g][:, c0 : c0 + N],
                            start=True,
                            stop=True,
                        )
                        nc.scalar.copy(zc[:, ct, c0 : c0 + N], ps[:])

            def conv_chunk(k):
                base = 512 * k + 1
                for co_t in range(4):
                    xt = p2x.tile([128, 1024], f32, tag="xt")
                    xv = xt[:].rearrange("p (n two) -> p two n", two=2)
                    for par in range(2):
                        ps = psC.tile([128, 512], f32, tag="psC")
                        for jj in range(5):
                            for ci_t in range(8):
                                nc.tensor.matmul(
                                    ps[:],
                                    lhsT=Mt[:, co_t, ci_t, par, jj],
                                    rhs=zc[:, ci_t, base + jj : base + jj + 512],
                                    start=(jj == 0 and ci_t == 0),
                                    stop=(jj == 4 and ci_t == 7),
                                )
                        nc.vector.tensor_scalar_add(
                            xv[:, par], ps[:], bh[:, co_t : co_t + 1]
                        )
                    nc.scalar.dma_start(
                        out=x_d.ap()[
                            co_t * 128 : co_t * 128 + 128,
                            1024 * k : 1024 * k + 1024,
                        ],
                        in_=xt[:],
                    )

            # All gathers emitted first (lowest scheduler priority) so the
            # serial gpsimd stream runs chunk-major/group-interleaved and
            # never reorders behind tensor work; tensor work is emitted in
            # consumption order. Tile inserts slice-level deps.
            for ci in range(len(G_CH)):
                gather_chunk(ci)
            cast_chunk(0)
            a_chunk(0)
            cast_chunk(1)
            a_chunk(1)
            cast_chunk(2)
            a_chunk(2)
            conv_chunk(0)
            cast_chunk(3)
            a_chunk(3)
            cast_chunk(4)
            a_chunk(4)
            conv_chunk(1)
            a_chunk(5)
            cast_chunk(5)
            a_chunk(6)
            conv_chunk(2)
            a_chunk(7)
            a_chunk(8)
            conv_chunk(3)
    nc.compile()
    return nc


# ---------------------------------------------------------------- entry
_CACHE = {}
_LAST_CORR = None


def _get_nc(dt_conv):
    if dt_conv not in _CACHE:
        _CACHE[dt_conv] = build_nc(dt_conv)
    return _CACHE[dt_conv]


def make_in_maps(inputs, dt_conv=DT_CONV):
    global _LAST_CORR
    codebooks = np.asarray(inputs["codebooks"], np.float32)
    scales = np.asarray(inputs["scales"], np.float32)
    W_out = np.asarray(inputs["W_out"], np.float32)
    b_out = np.asarray(inputs["b_out"], np.float32)
    W_up = np.asarray(inputs["W_up"], np.float32)
    b_up = np.asarray(inputs["b_up"], np.float32)
    W_head = np.asarray(inputs["W_head"], np.float32)
    b_head = np.asarray(inputs["b_head"], np.float32)
    indices = np.asarray(inputs["indices"])
    shared = prep_shared(codebooks, scales, W_out, b_out, W_up, b_up,
                         W_head, b_head)
    _LAST_CORR = host_corrections(indices, codebooks, scales, W_out, b_out,
                                  W_up, b_up, W_head, b_head)
    in_maps = []
    for core in range(8):
        m = dict(shared)
        m["idx"] = prep_core(indices, core)
        in_maps.append(m)
    return in_maps


def assemble(results, corr=None):
    corr = corr if corr is not None else _LAST_CORR
    out = np.empty((B, HEAD_OUT, 2 * L), np.float32)
    for core in range(8):
        b, half = core // 2, core % 2
        out[b, :, half * 2 * LLOC : (half + 1) * 2 * LLOC] = results[core]["x"]
    if corr is not None:
        left, right = corr
        out[:, :, 0:3] -= left
        out[:, :, 2 * L - 3 : 2 * L] -= right
    return out


def kernel(**inputs):
    nc = _get_nc(DT_CONV)
    in_maps = make_in_maps(inputs, DT_CONV)
    res = run_bass_kernel_spmd(nc, in_maps, list(range(8)))
    return assemble(res.results, _LAST_CORR)


# revision 20
# speedup vs baseline: 1.1452x; 1.0418x over previous
"""Trainium2 Bass kernel for nn_Decoder (VQ codebook decoder), fused-conv version.

Pipeline (per batch b): gather codebook entries by index, scale, sum over
quantizers, per-group linear projection -> zc [1024, L]; ConvTranspose1d
(1024->1024, k=4, s=2, SAME) -> zu; Conv1d (1024->512, k=7, SAME) -> x.

Key optimization: the upsample and head conv are linear maps, so they are
fused on the host into a 5-tap conv directly on zc, split by output parity:
  x[2n]   = sum_{j=-2..2} E_j zc[n+j] + b_fused
  x[2n+1] = sum_{j=-2..2} O_j zc[n+j] + b_fused
with E_j/O_j = sums of W_head[:,:,k] @ W_up[:,:,q]^T products ([512,1024]
each) and b_fused = b_head + sum_k W_head[:,:,k] b_up. This halves tensor
engine FLOPs (21.5 GF vs 47.3 GF per core) and removes the ze/zo HBM
round-trip.

The fused interior formula is exact except where the head conv window
touches zu frames outside [0, 2L): extending the upsample formula leaks
zu[-1] = W3^T zc[0] and zu[2L] = W0^T zc[L-1] (all other out-of-range zu
frames evaluate to 0 because zc[m]=0 outside [0,L), enforced on-device by
gathering a zero column for out-of-range indices). The 3 leftmost / 3
rightmost output columns per sequence are corrected exactly on the host,
including the (position-dependent) b_up validity terms.

Sharding: 8 cores = (batch b in 0..3) x (half of L). Each core computes
x[b, :, half*4096 : (half+1)*4096] from zc frames [l0-3, l1+2) (halo
recomputed locally; gathers are chunked and pipelined against the
projection matmuls and the fused-conv matmuls).
"""
import numpy as np
import ml_dtypes

import concourse.mybir as mybir
import concourse.tile as tile
from concourse import bacc
from concourse.bass_utils import run_bass_kernel_spmd

# structural constants (hardcoded per contest contract)
G, Q, C, D = 2, 8, 1024, 8
DIMS, DPG = 1024, 512
B, L = 4, 4096
HEAD_OUT = 512
LLOC = 2048          # frames per core
EFR = 2053           # zc frames [l0-3, l1+2)
NIDX = 2064          # gather count (mult of 16, >= EFR)

f32 = mybir.dt.float32
bf16 = mybir.dt.bfloat16
i16 = mybir.dt.int16

DT_CONV = "bf16"

# stage A psum chunks (<=512 cols per PSUM bank); conv chunks of 512.
# Finer chunks at the head so the first conv chunk can start early.
A_CH = [(0, 256), (256, 256), (512, 256), (768, 256), (1024, 256),
        (1280, 256), (1536, 256), (1792, 256), (2048, 5)]
# gather chunks in idx-columns (16 indices per col); X cols = 16*j0..16*(j0+nj)
G_CH = [(0, 16), (16, 16), (32, 16), (48, 16), (64, 32), (96, 33)]


# ---------------------------------------------------------------- host prep
def prep_shared(codebooks, scales, W_out, b_out, W_up, b_up, W_head, b_head):
    """Core-independent packed arrays (f32 gather table, bf16 weights)."""
    cbT = np.zeros((G, 128, C + 1), np.float32)
    W2 = np.zeros((G, 128, DPG), np.float32)
    for g in range(G):
        for q in range(Q):
            cbT[g, 16 * q : 16 * q + 8, :C] = codebooks[g, q].T
            cbT[g, 16 * q + 8, :C] = 1.0
            W2[g, 16 * q : 16 * q + 8] = scales[g, q][:, None] * W_out[g]
        W2[g, 8] = b_out[g]  # bias carrier: q=0's ones-row only
    W2pack = np.empty((128, G * DPG), np.float32)
    for g in range(G):
        W2pack[:, g * DPG : (g + 1) * DPG] = W2[g]

    # fused conv matrices: x[2n+p] = sum_j M[p,j] zc[n+j] + bfused
    W = [W_up[:, :, k].T.astype(np.float32) for k in range(4)]  # [zu, zc]
    H = [W_head[:, :, k].astype(np.float32) for k in range(7)]  # [x, zu]
    E = [
        H[0] @ W[1] + H[1] @ W[0],
        H[0] @ W[3] + H[1] @ W[2] + H[2] @ W[1] + H[3] @ W[0],
        H[2] @ W[3] + H[3] @ W[2] + H[4] @ W[1] + H[5] @ W[0],
        H[4] @ W[3] + H[5] @ W[2] + H[6] @ W[1],
        H[6] @ W[3],
    ]
    O = [
        H[0] @ W[0],
        H[0] @ W[2] + H[1] @ W[1] + H[2] @ W[0],
        H[1] @ W[3] + H[2] @ W[2] + H[3] @ W[1] + H[4] @ W[0],
        H[3] @ W[3] + H[4] @ W[2] + H[5] @ W[1] + H[6] @ W[0],
        H[5] @ W[3] + H[6] @ W[2],
    ]
    bfused = b_head + sum(H[k] @ b_up for k in range(7))
    # Mpack[ci_lo, co_t, ci_t, p, j, co_lo] = M[p][j][co_t*128+co_lo, ci_t*128+ci_lo]
    # (co_t outermost so each co-tile's weights are one contiguous DMA)
    Mpack = np.empty((128, 4, 8, 2, 5, 128), np.float32)
    for p, mats in enumerate((E, O)):
        for j in range(5):
            m4 = mats[j].reshape(4, 128, 8, 128)  # [co_t, co_lo, ci_t, ci_lo]
            Mpack[:, :, :, p, j, :] = m4.transpose(3, 0, 2, 1)
    bhead = bfused.reshape(4, 128).T  # [128, co_t]
    return dict(
        cbT=cbT.astype(np.float32),
        W2=W2pack.astype(ml_dtypes.bfloat16),
        M=Mpack.astype(ml_dtypes.bfloat16),
        bh=bhead.astype(np.float32),
    )


def prep_core(indices, core):
    """Per-core packed index array."""
    b, half = core // 2, core % 2
    l0 = half * LLOC
    ls = np.arange(NIDX) + l0 - 3
    valid = (ls >= 0) & (ls < L)
    lc = np.clip(ls, 0, L - 1)
    idx16 = np.empty((G, 128, NIDX // 16), np.int16)
    for g in range(G):
        for q in range(Q):
            v = indices[b, g * Q + q, lc]
            stream = np.where(valid & (v >= 0), v, C).astype(np.int16)
            idx16[g, 16 * q : 16 * q + 16] = stream.reshape(NIDX // 16, 16).T
    return idx16


def host_corrections(indices, codebooks, scales, W_out, b_out, W_up, b_up,
                     W_head, b_head):
    """Exact fix-ups for the 3 leftmost/rightmost output columns per batch.

    Returns (left [B, 512, 3], right [B, 512, 3]) to SUBTRACT from the
    assembled output at columns 0..2 and 2L-3..2L-1.
    """
    ii = indices.reshape(B, G, Q, L)
    H = [W_head[:, :, k].astype(np.float32) for k in range(7)]

    def zc_frame(b, l):
        zs = []
        for g in range(G):
            acc = np.zeros(D, np.float32)
            for q in range(Q):
                v = int(ii[b, g, q, l])
                if v >= 0:
                    acc += codebooks[g, q, v] * scales[g, q]
            zs.append(acc @ W_out[g] + b_out[g])
        return np.concatenate(zs)  # [1024]

    left = np.zeros((B, HEAD_OUT, 3), np.float32)
    right = np.zeros((B, HEAD_OUT, 3), np.float32)
    for b in range(B):
        vleft = W_up[:, :, 3].T @ zc_frame(b, 0)      # zu_ext[-1]
        vright = W_up[:, :, 0].T @ zc_frame(b, L - 1)  # zu_ext[2L]
        for t in range(3):
            hb = sum(H[k] for k in range(0, 3 - t)) @ b_up  # invalid f<0
            left[b, :, t] = H[2 - t] @ vleft + hb
        for s, t in enumerate(range(2 * L - 3, 2 * L)):
            hb = sum(H[k] for k in range(2 * L + 3 - t, 7)) @ b_up  # f>=2L
            right[b, :, s] = H[2 * L + 3 - t] @ vright + hb
    return left, right


# ---------------------------------------------------------------- bass build
def build_nc(dt_conv=DT_CONV, reps=1):
    dt = bf16
    nc = bacc.Bacc("TRN2", target_bir_lowering=False, debug=False)

    cbT_d = nc.dram_tensor("cbT", [G, 128, C + 1], f32, kind="ExternalInput")
    idx_d = nc.dram_tensor("idx", [G, 128, NIDX // 16], i16, kind="ExternalInput")
    W2_d = nc.dram_tensor("W2", [128, G * DPG], dt, kind="ExternalInput")
    M_d = nc.dram_tensor("M", [128, 4, 8, 2, 5, 128], dt, kind="ExternalInput")
    bh_d = nc.dram_tensor("bh", [128, 4], f32, kind="ExternalInput")
    x_d = nc.dram_tensor("x", [HEAD_OUT, 2 * LLOC], f32, kind="ExternalOutput")

    with tile.TileContext(nc) as tc:
      for rep in range(reps):
        with (
            tc.tile_pool(name=f"p1c{rep}", bufs=1) as p1c,
            tc.tile_pool(name=f"p2x{rep}", bufs=3) as p2x,
            tc.tile_pool(name=f"psA{rep}", bufs=2, space="PSUM") as psA,
            tc.tile_pool(name=f"psC{rep}", bufs=6, space="PSUM") as psC,
        ):
            cbT = [p1c.tile([128, C + 1], f32, name=f"cbT{g}") for g in range(G)]
            idxs = [p1c.tile([128, NIDX // 16], i16, name=f"idxs{g}") for g in range(G)]
            W2t = p1c.tile([128, G * DPG], dt, name="W2t")
            X = [p1c.tile([128, NIDX], f32, name=f"X{g}") for g in range(G)]
            Xb = [p1c.tile([128, NIDX], dt, name=f"Xb{g}") for g in range(G)]
            zc = p1c.tile([128, 8, EFR], dt, name="zc")
            Mt = p1c.tile([128, 4, 8, 2, 5, 128], dt, name="Mt")
            bh = p1c.tile([128, 4], f32, name="bh")

            # idx/table first: the gathers are the serial critical producer;
            # the big fused-weight DMA queues behind them (needed ~35us in).
            for g in range(G):
                nc.sync.dma_start(out=idxs[g][:], in_=idx_d.ap()[g])
                nc.sync.dma_start(out=cbT[g][:], in_=cbT_d.ap()[g])
            nc.sync.dma_start(out=W2t[:], in_=W2_d.ap())
            nc.sync.dma_start(out=bh[:], in_=bh_d.ap())
            for co_t in range(4):
                nc.sync.dma_start(out=Mt[:, co_t], in_=M_d.ap()[:, co_t])

            def gather_chunk(ci):
                j0, nj = G_CH[ci]
                x0, nx = 16 * j0, 16 * nj
                for g in range(G):
                    nc.gpsimd.ap_gather(
                        X[g][:, x0 : x0 + nx],
                        cbT[g][:],
                        idxs[g][:, j0 : j0 + nj],
                        channels=128,
                        num_elems=C + 1,
                        d=1,
                        num_idxs=nx,
                    )

            def cast_chunk(ci):
                # emitted in consumption order, NOT with the gathers: engines
                # run their program in-order, so a cast waiting on a late
                # gather must not sit ahead of conv psum drains in the
                # vector queue (it would stall the psC ring -> tensor).
                j0, nj = G_CH[ci]
                x0, nx = 16 * j0, 16 * nj
                for g in range(G):
                    nc.vector.tensor_copy(
                        Xb[g][:, x0 : x0 + nx], X[g][:, x0 : x0 + nx]
                    )

            def a_chunk(ai):
                c0, N = A_CH[ai]
                for g in range(G):
                    for et in range(4):
                        ct = g * 4 + et
                        ps = psA.tile([128, N], f32, tag="psA")
                        nc.tensor.matmul(
                            ps[:],
                            lhsT=W2t[:, g * DPG + et * 128 : g * DPG + et * 128 + 128],
                            rhs=Xb[g][:, c0 : c0 + N],
                            start=True,
                            stop=True,
                        )
                        nc.scalar.copy(zc[:, ct, c0 : c0 + N], ps[:])

            def conv_chunk(k):
                base = 512 * k + 1
                for co_t in range(4):
                    xt = p2x.tile([128, 1024], f32, tag="xt")
                    xv = xt[:].rearrange("p (n two) -> p two n", two=2)
                    for par in range(2):
                        ps = psC.tile([128, 512], f32, tag="psC")
                        for jj in range(5):
                            for ci_t in range(8):
                                nc.tensor.matmul(
                                    ps[:],
                                    lhsT=Mt[:, co_t, ci_t, par, jj],
                                    rhs=zc[:, ci_t, base + jj : base + jj + 512],
                                    start=(jj == 0 and ci_t == 0),
                                    stop=(jj == 4 and ci_t == 7),
                                )
                        nc.vector.tensor_scalar_add(
                            xv[:, par], ps[:], bh[:, co_t : co_t + 1]
                        )
                    nc.scalar.dma_start(
                        out=x_d.ap()[
                            co_t * 128 : co_t * 128 + 128,
                            1024 * k : 1024 * k + 1024,
                        ],
                        in_=xt[:],
                    )

            # All gathers emitted first (lowest scheduler priority) so the
            # serial gpsimd stream runs chunk-major/group-interleaved and
            # never reorders behind tensor work; tensor work is emitted in
            # consumption order. Tile inserts slice-level deps.
            for ci in range(len(G_CH)):
                gather_chunk(ci)
            cast_chunk(0)
            a_chunk(0)
            cast_chunk(1)
            a_chunk(1)
            cast_chunk(2)
            a_chunk(2)
            conv_chunk(0)
            cast_chunk(3)
            a_chunk(3)
            cast_chunk(4)
            a_chunk(4)
            conv_chunk(1)
            a_chunk(5)
            cast_chunk(5)
            a_chunk(6)
            conv_chunk(2)
            a_chunk(7)
            a_chunk(8)
            conv_chunk(3)
    nc.compile()
    return nc


# ---------------------------------------------------------------- entry
_CACHE = {}
_LAST_CORR = None


def _get_nc(dt_conv):
    if dt_conv not in _CACHE:
        _CACHE[dt_conv] = build_nc(dt_conv)
    return _CACHE[dt_conv]


def make_in_maps(inputs, dt_conv=DT_CONV):
    global _LAST_CORR
    codebooks = np.asarray(inputs["codebooks"], np.float32)
    scales = np.asarray(inputs["scales"], np.float32)
    W_out = np.asarray(inputs["W_out"], np.float32)
    b_out = np.asarray(inputs["b_out"], np.float32)
    W_up = np.asarray(inputs["W_up"], np.float32)
    b_up = np.asarray(inputs["b_up"], np.float32)
    W_head = np.asarray(inputs["W_head"], np.float32)
    b_head = np.asarray(inputs["b_head"], np.float32)
    indices = np.asarray(inputs["indices"])
    shared = prep_shared(codebooks, scales, W_out, b_out, W_up, b_up,
                         W_head, b_head)
    _LAST_CORR = host_corrections(indices, codebooks, scales, W_out, b_out,
                                  W_up, b_up, W_head, b_head)
    in_maps = []
    for core in range(8):
        m = dict(shared)
        m["idx"] = prep_core(indices, core)
        in_maps.append(m)
    return in_maps


def assemble(results, corr=None):
    corr = corr if corr is not None else _LAST_CORR
    out = np.empty((B, HEAD_OUT, 2 * L), np.float32)
    for core in range(8):
        b, half = core // 2, core % 2
        out[b, :, half * 2 * LLOC : (half + 1) * 2 * LLOC] = results[core]["x"]
    if corr is not None:
        left, right = corr
        out[:, :, 0:3] -= left
        out[:, :, 2 * L - 3 : 2 * L] -= right
    return out


def kernel(**inputs):
    nc = _get_nc(DT_CONV)
    in_maps = make_in_maps(inputs, DT_CONV)
    res = run_bass_kernel_spmd(nc, in_maps, list(range(8)))
    return assemble(res.results, _LAST_CORR)
